# revision 35
# baseline (speedup 1.0000x reference)
"""Causal self-attention (nn_CausalAttention), TP-sharded Bass kernel
for 8 Trainium2 NeuronCores.

Contract: kernel(x, w_qkv, w_out) takes the FULL fp32 inputs
(x [1,4096,1024], w_qkv [3072,1024], w_out [1024,1024]) and returns the
FULL fp32 output [1,4096,1024].

Sharding: tensor-parallel over heads -- 16 heads / 8 cores = 2 heads per
core. qkv weights are column-sharded, w_out row-sharded; each core
computes a full-shape partial output (transposed), the host sums the 8
partials and transposes back.

Per-core kernel (all matmul operands fp16, PSUM accumulation fp32; fp32
matmuls are 2-pass on the TRN2 PE and ~3x slower, while fp16 keeps the
output max-abs error at ~6e-4 of the output scale):
  - proj: qT/kT/vT [128,4096] dim-major; v PE-transposed to natural
    layout with a ones column appended (so the PV matmul also emits the
    softmax denominators). Projections of chunk j+2 stream through a
    dedicated PSUM bank interleaved into chunk j's attention jobs.
  - head-packed score slots: one [128,1024] PSUM tile per skv tile holds
    h0's scoresT in cols 0:512 (bank A) and h1's in 512:1024 (bank B).
    The two K=64 score matmuls hit different PE row groups AND different
    banks, so they run concurrently (2x), and a single exp ACTIVATE
    covers both heads and frees the slot at once (no h0/h1 stagger).
  - exp on ACT with the 1/sqrt(hd) scale fused; score matmuls + exp are
    emitted at high scheduler priority so they preempt fill work (proj/
    outproj) the moment a PSUM slot frees -- the exp stream is what
    keeps the bottleneck engines dense across chunk boundaries.
  - causal masking via precomputed 0/1 tiles on the diagonal blocks,
    fully-masked columns skipped in the matmuls.
  - o_augT[d+1,sq] += v_aug.T @ expT accumulated over skv tiles in PSUM,
    software-pipelined so the ACT exp stream never waits on a PSUM slot.
  - normalization via fast-reciprocal + ones-outer-product broadcast
    matmul, deferred and interleaved into the next chunk's score stream.
  - out-projection partialT[Dm,sq] = woT-tile.T @ oT, drained into a
    per-chunk staging buffer and written with one batched DMA per chunk
    (per-tile DMAs only for the last chunk so transfers overlap).
  - host-shuffled input layouts so weights/x each land with a handful of
    large contiguous DMAs (descriptor issue is ~630ns each on the sync
    engine, and sub-2KB partition fragments halve HBM bandwidth).
"""

import numpy as np

import concourse.bass as bass
import concourse.mybir as mybir
import concourse.tile as tile
from concourse import bacc
from concourse.masks import make_identity
from concourse.bass_utils import run_bass_kernel_spmd

F32 = mybir.dt.float32
F16 = mybir.dt.float16
U16 = mybir.dt.uint16

# Schraudolph exp-approx constants (fp16 bit trick on DVE):
#   exp(s) ~= bitcast_fp16(uint16(round(s * SCH_A + SCH_B)))
# max rel err ~3% in the live range; negatives saturate to +0.0 via the
# uint16 convert.  Used to offload part of the exp stream from the
# saturated ACT engine to the DVE in the late (ACT-bound) chunks.
SCH_A = 1024.0 / float(np.log(2.0))
SCH_B = 15360.0 - 0.0434 * 1024.0

S = 4096        # sequence length
D = 1024        # model dim
HD = 64         # head dim
NH_LOC = 2      # heads per core
DL = HD * NH_LOC  # local dims = 128
SCALE = HD ** -0.5
VW = 80         # padded v_aug row stride (32B-aligned for dma transpose)

N_KC = D // 128       # 8 contraction tiles for projections
N_J = S // 512        # 8 sq chunks
N_I = S // 128        # 32 skv tiles


def build_kernel(n_cores=8):
    nc = bacc.Bacc("TRN2", target_bir_lowering=False, debug=False,
                   num_devices=n_cores)

    # host-shuffled layouts: one big DMA per tensor instead of per-tile
    # (the sync engine issues descriptors at ~630ns each -- 121 small DMAs
    # cost ~73us of serial issue time).
    xS = nc.dram_tensor("xS", [128, N_J, N_KC, 512], F16, kind="ExternalInput")
    wqS = nc.dram_tensor("wqS", [128, N_KC, DL], F16, kind="ExternalInput")
    wkS = nc.dram_tensor("wkS", [128, N_KC, DL], F16, kind="ExternalInput")
    wvS = nc.dram_tensor("wvS", [128, N_KC, DL], F16, kind="ExternalInput")
    woT = nc.dram_tensor("woT", [DL, D], F16, kind="ExternalInput")
    # output: [p, t, j, c] with full dim d = t*128+p, s = 512*j+c
    outV = nc.dram_tensor("outV", [128, N_KC, N_J, 512], F16,
                          kind="ExternalOutput")

    with tile.TileContext(nc) as tc:
        build_body(tc, xS, wqS, wkS, wvS, woT, outV)

    nc.compile()
    return nc


def build_body(tc, xS, wqS, wkS, wvS, woT, outV):
    nc = tc.nc

    with tc.tile_pool(name="persist", bufs=1) as persist:
        # persistent SBUF tensors
        qT = persist.tile([128, S], F16, tag="qT")
        kT = persist.tile([128, S], F16, tag="kT")
        vT = persist.tile([128, S], F16, tag="vT")
        v_aug = [persist.tile([128, N_I, VW], F16, tag=f"vaug{h}",
                              name=f"vaug{h}") for h in range(2)]
        # chunk-major x layout: per-chunk DMA pieces land as one contiguous
        # 8KB run per partition (1KB-fragment transfers only reach half
        # the HBM bandwidth)
        x_sb = persist.tile([128, N_J, N_KC, 512], F16, tag="x_sb")
        wq_sb = persist.tile([128, N_KC, DL], F16, tag="wq")
        wk_sb = persist.tile([128, N_KC, DL], F16, tag="wk")
        wv_sb = persist.tile([128, N_KC, DL], F16, tag="wv")
        wo_sb = persist.tile([128, D], F16, tag="wo")
        ones_row = persist.tile([1, HD], F16, tag="ones")
        ident = persist.tile([128, 128], F16, tag="ident")
        masks = persist.tile([128, 4, 512], F16, tag="masks")

        make_identity(nc, ident[:])
        nc.vector.memset(ones_row[:], 1.0)
        nc.vector.memset(v_aug[0][:, :, HD], 1.0)
        nc.vector.memset(v_aug[1][:, :, HD], 1.0)
        nc.vector.memset(masks[:], 1.0)
        for r in range(4):
            # keep (=1.0) where skv <= sq i.e. f - p - 128*r >= 0
            nc.gpsimd.affine_select(
                out=masks[:, r, :], in_=masks[:, r, :],
                pattern=[[1, 512]], base=-128 * r, channel_multiplier=-1,
                compare_op=mybir.AluOpType.is_ge, fill=0.0,
            )

        # ---- fused pipeline: projections of chunk j+2 stream through a
        # dedicated PSUM bank as deferred items inside the attention job
        # stream, so the ACT exp stream starts ~50us earlier ----
        with (
            tc.tile_pool(name="sc_ps", bufs=2, space="PSUM") as sc_ps,
            tc.tile_pool(name="pv_ps", bufs=1, space="PSUM") as pv_ps,
            tc.tile_pool(name="op_ps", bufs=1, space="PSUM") as op_ps,
            tc.tile_pool(name="pj_ps", bufs=1, space="PSUM") as pj_ps,
            tc.tile_pool(name="exp_sb", bufs=8) as exp_pool,
            tc.tile_pool(name="att_sb", bufs=6) as att_sb,
            tc.tile_pool(name="out_sb", bufs=3) as out_pool,
        ):
            # DMA issue order tuned so the first q/k-projection matmuls can
            # start as early as possible: weights, the first 512 x columns,
            # then the bulk of x -- all as big multi-tile transfers
            nc.sync.dma_start(out=wq_sb[:], in_=wqS[:])
            nc.sync.dma_start(out=wk_sb[:], in_=wkS[:])
            nc.sync.dma_start(out=x_sb[:, 0, 0:4], in_=xS[:, 0, 0:4])
            nc.sync.dma_start(out=x_sb[:, 0, 4:8], in_=xS[:, 0, 4:8])
            nc.sync.dma_start(out=wv_sb[:], in_=wvS[:])
            nc.sync.dma_start(out=wo_sb[:], in_=woT[:])
            nc.sync.dma_start(out=x_sb[:, 1, 0:4], in_=xS[:, 1, 0:4])
            nc.sync.dma_start(out=x_sb[:, 1, 4:8], in_=xS[:, 1, 4:8])
            for lo, hi in ((2, 5), (5, N_J)):
                nc.sync.dma_start(out=x_sb[:, lo:hi], in_=xS[:, lo:hi])

            # warm up the PE (HAM clock gate) while the DMAs land
            warm_ps = op_ps.tile([128, 128], F32, tag="op", name="warm")
            for _ in range(40):
                nc.tensor.matmul(warm_ps[:], ident[:], ident[:],
                                 start=True, stop=True)

            def proj_part(j, which, use_op=False):
                """one accumulation sweep (q, k or v) of chunk j through
                the single pj PSUM bank; v also transposes into v_aug.
                The prologue sweeps (no attention work to hide the
                PSUM-drain copy) alternate into the idle op bank."""
                sl = bass.ts(j, 512)
                w_sb, dst = {"q": (wq_sb, qT), "k": (wk_sb, kT),
                             "v": (wv_sb, vT)}[which]
                if use_op:
                    ps = op_ps.tile([128, 512], F32, tag="op", name="pjo")
                else:
                    ps = pj_ps.tile([128, 512], F32, tag="pj", name="pj")
                for kc in range(N_KC):
                    nc.tensor.matmul(ps[:], w_sb[:, kc, :], x_sb[:, j, kc, :],
                                     start=kc == 0, stop=kc == N_KC - 1)
                nc.vector.tensor_copy(dst[:, sl], ps[:])
                if which == "v":
                    for ii in range(4):
                        i = 4 * j + ii
                        t_ps = op_ps.tile([128, 128], F16, tag="op",
                                          name="t_ps")
                        nc.tensor.transpose(t_ps[:], vT[:, bass.ts(i, 128)],
                                            ident[:])
                        nc.vector.tensor_copy(v_aug[0][:, i, 0:HD],
                                              t_ps[:, 0:HD])
                        nc.vector.tensor_copy(v_aug[1][:, i, 0:HD],
                                              t_ps[:, HD:128])

            def do_proj(j):
                for n, which in enumerate(("q", "k", "v")):
                    proj_part(j, which, use_op=(n == 1))

            # deferred work items (projections of later chunks + previous
            # chunk's normalization/out-projection), interleaved into the
            # attention job stream
            pending = []

            def drain_pending(k=None):
                n = len(pending) if k is None else min(k, len(pending))
                for _ in range(n):
                    pending.pop(0)()

            def emit_attention(j):
                sl = bass.ts(j, 512)
                n_i = 4 * j + 4
                pv = [pv_ps.tile([HD + 1, 512], F32, tag=f"pv{h}",
                                 name=f"pv{h}") for h in range(2)]
                oc = att_sb.tile([128, 512], F16, tag="oc")

                # head-packed score slots: one [128,1024] PSUM tile per skv
                # tile i holds h0's scoresT in cols 0:512 (bank A) and h1's
                # in cols 512:1024 (bank B).  The two score matmuls hit
                # different row groups (0 / 64) AND different PSUM banks, so
                # the PE runs them concurrently; one exp ACTIVATE covers
                # both heads and frees the whole slot at once, keeping the
                # next pair adjacent (no h0/h1 stagger).
                def emit_scores(i):
                    # high scheduler priority: the exp stream feeds the
                    # bottleneck ACT engine -- score matmuls must preempt
                    # queued fill work (proj/outproj) the moment their PSUM
                    # slot frees, especially across chunk boundaries.
                    with tc.high_priority(offset=4000):
                        sc = sc_ps.tile([128, 1024], F32, tag="sc", name="sc")
                        rr = i - 4 * j
                        lo = 128 * rr if 0 <= rr < 4 else 0
                        for h in range(2):
                            hsl = slice(h * HD, (h + 1) * HD)
                            nc.tensor.matmul(
                                sc[:, 512 * h + lo:512 * (h + 1)],
                                kT[hsl, bass.ts(i, 128)],
                                qT[hsl, 512 * j + lo:512 * (j + 1)],
                                start=True, stop=True,
                            )
                        ex = exp_pool.tile([128, 1024], F16, tag="ex",
                                           name="ex")
                        if False and lo == 0 and j >= 5 and i % 3 == 0:
                            # ACT-bound late chunks: offload this tile's exp
                            # to the DVE via the Schraudolph bit trick
                            nc.vector.tensor_scalar(
                                out=ex[:].bitcast(U16), in0=sc[:],
                                scalar1=SCH_A * SCALE, scalar2=SCH_B,
                                op0=mybir.AluOpType.mult,
                                op1=mybir.AluOpType.add,
                            )
                        elif lo == 0:
                            nc.scalar.activation(
                                ex[:], sc[:],
                                mybir.ActivationFunctionType.Exp,
                                scale=SCALE)
                        else:
                            for h in range(2):
                                s = slice(512 * h + lo, 512 * (h + 1))
                                nc.scalar.activation(
                                    ex[:, s], sc[:, s],
                                    mybir.ActivationFunctionType.Exp,
                                    scale=SCALE)
                    return (i, ex)

                def emit_pv(job):
                    i, ex = job
                    rr = i - 4 * j
                    lo = 128 * rr if 0 <= rr < 4 else 0
                    if 0 <= rr < 4:
                        for h in range(2):
                            s = slice(512 * h + 128 * rr, 512 * h + 128 * rr + 128)
                            nc.vector.tensor_mul(
                                ex[:, s], ex[:, s],
                                masks[:, rr, 128 * rr:128 * rr + 128],
                            )
                    for h in range(2):
                        nc.tensor.matmul(
                            pv[h][:, lo:512],
                            v_aug[h][:, i, 0:HD + 1],
                            ex[:, 512 * h + lo:512 * (h + 1)],
                            start=(i == 0), stop=(i == n_i - 1),
                            skip_group_check=True,
                        )

                prev = None
                for i in range(n_i):
                    job = emit_scores(i)
                    if prev is not None:
                        emit_pv(prev)
                        drain_pending(2)
                    prev = job
                emit_pv(prev)
                # queue the projection of chunk j+2 (streams through the
                # pj PSUM bank while chunk j+1's attention runs)
                if j + 2 < N_J:
                    for which in ("q", "k", "v"):
                        pending.append(
                            lambda w=which, jj=j + 2: proj_part(jj, w))

                # ---- drain the PV psum right away (DVE ops, free the pv
                # slots before chunk j+1's PV matmuls need them) ----
                o_sbs, s_sbs = [], []
                for h in range(2):
                    s_sb = att_sb.tile([1, 512], F32, tag="s_sb", name="s_sb")
                    nc.vector.tensor_copy(s_sb[:], pv[h][HD:HD + 1, :])
                    s_sbs.append(s_sb)
                    o_sb = att_sb.tile([HD, 512], F16, tag="o_sb", name="o_sb")
                    if j == N_J - 1 and h == 1:
                        nc.scalar.copy(o_sb[:], pv[h][0:HD, :])
                    else:
                        nc.vector.tensor_copy(o_sb[:], pv[h][0:HD, :])
                    o_sbs.append(o_sb)

                # ---- queue the rest of this chunk's tail work ----
                recips = []

                def norm_dve(recips=recips, s_sbs=s_sbs):
                    # DVE-only part: reciprocals of the softmax sums
                    for h in range(2):
                        recip = att_sb.tile([1, 512], F32, tag="recip",
                                            name="recip")
                        nc.vector.reciprocal_approx_fast(recip[:], s_sbs[h][:])
                        r16 = att_sb.tile([1, 512], F16, tag="recip16",
                                          name="recip16")
                        nc.vector.tensor_copy(r16[:], recip[:])
                        recips.append(r16)

                def norm_mul(oc=oc, recips=recips, o_sbs=o_sbs, j=j):
                    for h in range(2):
                        if j < N_J - 1:
                            # off the critical path: broadcast the recip row
                            # on the idle gpsimd engine; the multiply then
                            # runs all-SBUF fp16 on DVE
                            bcs = att_sb.tile([HD, 512], F16, tag="bcs",
                                              name="bcs")
                            nc.gpsimd.partition_broadcast(bcs[:], recips[h][:])
                            nc.vector.tensor_mul(oc[h * HD:(h + 1) * HD, :],
                                                 o_sbs[h][:], bcs[:])
                            continue
                        # final chunk: PE outer-product, double-banked so the
                        # serial tail chain pipelines
                        if h == 1:
                            bc = pj_ps.tile([HD, 512], F32, tag="pj",
                                            name="bc2")
                        else:
                            bc = op_ps.tile([HD, 512], F32, tag="op",
                                            name="bc")
                        nc.tensor.matmul(bc[:], ones_row[:], recips[h][:],
                                         start=True, stop=True)
                        nc.vector.tensor_mul(oc[h * HD:(h + 1) * HD, :],
                                             o_sbs[h][:], bc[:])

                osb = out_pool.tile([128, N_KC, 512], F16, tag="ot",
                                    name="osb")

                def outproj(t, j=j, oc=oc, osb=osb):
                    # chunks past the last projection sweep can double-buffer
                    # the out-projection through the idle pj bank
                    if j >= N_J - 3 and t % 2 == 1:
                        op = pj_ps.tile([128, 512], F32, tag="pj", name="op2")
                    else:
                        op = op_ps.tile([128, 512], F32, tag="op", name="op")
                    nc.tensor.matmul(op[:], wo_sb[:, bass.ts(t, 128)], oc[:],
                                     start=True, stop=True)
                    if j == N_J - 1 and t % 2 == 1:
                        # the exp stream is done by now: give half the final
                        # drain copies to the idle ACT engine
                        nc.scalar.copy(osb[:, t, :], op[:])
                    else:
                        nc.vector.tensor_copy(osb[:, t, :], op[:])
                    if j == N_J - 1:
                        # last chunk: per-tile DMAs so the transfers overlap
                        # the remaining out-projection work instead of one
                        # big transfer after the final drain
                        nc.sync.dma_start(out=outV[:, t, j, :],
                                          in_=osb[:, t, :])
                    elif t == N_KC - 1:
                        # one batched DMA for the whole chunk
                        nc.sync.dma_start(out=outV[:, :, j, :], in_=osb[:])

                pending.append(norm_dve)
                pending.append(norm_mul)
                pending.append(lambda: None)
                for t in range(N_KC):
                    pending.append(lambda t=t: outproj(t))

            do_proj(0)
            # chunk 1's projections drain inside attention(0)'s job stream
            # (attention(0) only reads chunk 0's k/v), so the exp stream
            # starts as soon as the first chunk is projected
            for which in ("q", "k", "v"):
                pending.append(lambda w=which: proj_part(1, w))
            for j in range(N_J):
                emit_attention(j)
            drain_pending()


# ---------------- host-side sharding / unsharding ----------------

def _shuffle_w(wT):
    """[D, DL] -> sbuf image [128, N_KC, DL]: row p holds tiles kc with
    original row kc*128+p."""
    return np.ascontiguousarray(
        wT.reshape(N_KC, 128, DL).transpose(1, 0, 2))


def shard_inputs(x, w_qkv, w_out, n_cores=8):
    """Full inputs -> per-core in_maps (big-DMA shuffled layouts)."""
    x2 = np.asarray(x, np.float32).reshape(S, D)
    xT_h = x2.T.astype(np.float16)                            # [D, S]
    xS_h = np.ascontiguousarray(
        xT_h.reshape(N_KC, 128, N_J, 512)
            .transpose(1, 2, 0, 3))                   # [128, N_J, N_KC, 512]
    w_qkv = np.asarray(w_qkv, np.float32)
    w_out = np.asarray(w_out, np.float32)
    in_maps = []
    for c in range(n_cores):
        lo, hi = c * DL, (c + 1) * DL
        in_maps.append({
            "xS": xS_h,
            "wqS": _shuffle_w(w_qkv[lo:hi, :].T.astype(np.float16)),
            "wkS": _shuffle_w(w_qkv[D + lo:D + hi, :].T.astype(np.float16)),
            "wvS": _shuffle_w(w_qkv[2 * D + lo:2 * D + hi, :].T.astype(np.float16)),
            "woT": np.ascontiguousarray(w_out[:, lo:hi].T.astype(np.float16)),
        })
    return in_maps


def unshard_outputs(results):
    acc = results[0]["outV"].astype(np.float32)
    for r in results[1:]:
        acc += r["outV"].astype(np.float32)
    # [128, t, j, c] -> full [D, S]: d = t*128+p, s = 512*j+c
    full = acc.transpose(1, 0, 2, 3).reshape(D, S)
    return np.ascontiguousarray(full.T).reshape(1, S, D)


# ---------------- public entry point ----------------

_NC_CACHE = []


def _get_nc():
    if not _NC_CACHE:
        _NC_CACHE.append(build_kernel())
    return _NC_CACHE[0]


def kernel(x, w_qkv, w_out):
    nc = _get_nc()
    in_maps = shard_inputs(x, w_qkv, w_out)
    res = run_bass_kernel_spmd(nc, in_maps, list(range(8)))
    out = unshard_outputs(res.results)
    return out.astype(np.float32)

